# revision 28
# baseline (speedup 1.0000x reference)
"""CGNet (NNConv + GRU message passing) Trainium2 kernel, 8-way graph-parallel.

Sharding: graphs (and their nodes via the sorted batch vector) are
partitioned across 8 cores; edges live on the core owning their dst node.
Per-edge weight blocks Wedge = edgeMLP(edge_attr) are built once on device
(TensorE, bf16) and staged in HBM. Each iteration: AllGather the node
table, dma_gather out[src] rows, multiply Wedge tiles by the replicated
source vectors on VectorE, then reduce over the input dim AND scatter to
per-tile dst slots in one TensorE pass (one-hot stationary + PSUM revisit
accumulation), dma_scatter_add unique dst rows into an HBM agg table
pre-filled with b_conv, and run the GRU cell per node tile in fp32.
"""
import sys

sys.path.insert(0, "/opt/trn_rl_repo")

import numpy as np
import ml_dtypes

import concourse.bass as bass
import concourse.mybir as mybir
import concourse.tile as tile
from concourse import bacc
from concourse.bass_utils import run_bass_kernel_spmd
from concourse.masks import make_identity

f32 = mybir.dt.float32
bf16 = mybir.dt.bfloat16
i16 = mybir.dt.int16
AF = mybir.ActivationFunctionType
ALU = mybir.AluOpType

N_NODES = 25000
N_EDGES = 50000
NUM_ATOMS = 26
HID = 64
EMB = 64
N_GRAPHS = 64
NUM_NN_ITER = 3
NCORE = 8
GPC = N_GRAPHS // NCORE

NT = 28
NPAD = NT * 128              # 3584 padded nodes per core
ET = 52
EPAD = ET * 128              # 6656 padded edges per core
NTAB = NPAD * NCORE
AGG_T = NT + 1               # +1 trash tile for pad slots
AGG_ROWS = AGG_T * 128
IDXC = EPAD // 16
RO_W = 16
FEAT = EMB + NUM_ATOMS


def _wrap_idx(idx):
    a = np.asarray(idx, dtype=np.int64)
    assert a.shape[0] == EPAD
    w = a.reshape(IDXC, 16).T.astype(np.int16)
    return np.tile(w, (8, 1))


def _tile_major(rows, ntiles):
    d = rows.shape[1]
    return np.ascontiguousarray(rows.reshape(ntiles, 128, d).transpose(1, 0, 2))


def _build_program(phases=4):
    nc = bacc.Bacc("TRN2", target_bir_lowering=False, debug=False,
                   num_devices=NCORE)

    def par(name, shape, dtype, out=False):
        return nc.declare_dram_parameter(name, list(shape), dtype, isOutput=out)

    eaT_in = par("eaT", (4, EPAD), f32)
    idx_in = par("idx2", (128, 2 * IDXC), i16)     # [gather | scatter]
    srel_in = par("srel", (128, ET, 128), bf16)
    xT_in = par("xT", (NUM_ATOMS, NPAD), f32)
    xloc_in = par("xloc", (128, NT, NUM_ATOMS), f32)
    sb_in = par("sbro", (128, NT, RO_W), f32)
    win_in = par("w_in", (NUM_ATOMS, HID), f32)
    we1_in = par("we1", (4, 128), f32)
    be1_in = par("be1_c", (128, 1), f32)
    we2_in = par("we2", (128, 4096), bf16)
    be2_in = par("be2_r", (1, 4096), f32)
    wpack_in = par("wpack", (HID, 576), f32)       # [Wroot|Wo1|Wo2|WihT|WhhT]
    bpack_in = par("bpack", (1, 576), f32)         # [b_in|b_ih|b_hh|bo1|bo2]
    bconv_in = par("bconv_rep", (128, HID), f32)
    wp_in = par("wp_rep", (RO_W, FEAT), f32)
    bp_in = par("bp_rep", (RO_W, 1), f32)

    feat_out = par("feat_out", (NPAD, FEAT), f32, out=True)
    ratio_out = par("ratio_out", (RO_W, 1), f32, out=True)
    dbg_out = par("dbg_h", (NPAD, HID), f32, out=True)

    wedge_hbm = nc.dram_tensor("wedge_hbm", [ET, 128, 4096], bf16)
    local_out = nc.dram_tensor("local_out", [NPAD, HID], f32)
    table = nc.dram_tensor("table", [NTAB, HID], f32, addr_space="Shared")
    agg_hbm = nc.dram_tensor("agg_hbm", [AGG_ROWS, HID], f32)

    def node_rows_ap(dram, ntiles, d):
        return bass.AP(tensor=dram, offset=0,
                       ap=[[d, 128], [128 * d, ntiles], [1, d]])

    with tile.TileContext(nc) as tc:
        with (
            tc.tile_pool(name="persist", bufs=1) as P,
            tc.tile_pool(name="wts", bufs=1) as W,
            tc.tile_pool(name="loop", bufs=2) as L,
            tc.tile_pool(name="sm2", bufs=2) as S,
            tc.tile_pool(name="sc8", bufs=8) as SC,
        ):
            def load(pool, param, shape, dtype):
                t = pool.tile(list(shape), dtype, tag=param.name)
                nc.sync.dma_start(t[:], param[:])
                return t

            B_cm = tc.tile_pool(name="build", bufs=1)
            B = B_cm.__enter__()
            xT = load(B, xT_in, (NUM_ATOMS, NPAD), f32)
            w_in = load(B, win_in, (NUM_ATOMS, HID), f32)
            we1 = load(B, we1_in, (4, 128), f32)
            be1_c = load(B, be1_in, (128, 1), f32)
            we2 = load(B, we2_in, (128, 4096), bf16)
            be2_r = load(B, be2_in, (1, 4096), f32)
            wpack = load(W, wpack_in, (HID, 576), f32)
            bpack = load(W, bpack_in, (1, 576), f32)
            w_root = wpack[:, 0:64]
            wo1 = wpack[:, 64:128]
            wo2 = wpack[:, 128:192]
            w_ihT = wpack[:, 192:384]
            w_hhT = wpack[:, 384:576]
            b_in_r = bpack[:, 0:64]
            b_ih_r = bpack[:, 64:256]
            b_hh_r = bpack[:, 256:448]
            bo1_r = bpack[:, 448:512]
            bo2_r = bpack[:, 512:576]

            # constants: [0:128]=identity, col 128 = ones col, row0[132:260] = ones row
            bigc = P.tile([128, 260], f32)
            nc.gpsimd.memset(bigc[:], 0.0)
            make_identity(nc, bigc[:, 0:128], nomemset=True)
            nc.gpsimd.memset(bigc[:, 128:129], 1.0)
            nc.gpsimd.memset(bigc[0:1, 132:260], 1.0)
            ident = bigc[:, 0:128]
            ones_c = bigc[:, 128:129]
            ones_r = bigc[0:1, 132:260]

            h_a = P.tile([128, NT, HID], f32)

            # ===== phase 1: h2T and initial node state =====
            psA_cm = tc.tile_pool(name="psA", bufs=2, space="PSUM")
            PSA = psA_cm.__enter__()
            h2T = B.tile([128, EPAD], bf16)
            for j in range(EPAD // 512 if phases >= 0.7 else 0):
                eas = B.tile([4, 512], f32, tag="eas")
                nc.sync.dma_start(eas[:], eaT_in[:, j * 512:(j + 1) * 512])
                ph = PSA.tile([128, 512], f32, tag="ph2")
                nc.tensor.matmul(ph[:], we1[:], eas[:], start=True, stop=True)
                nc.scalar.activation(h2T[:, j * 512:(j + 1) * 512], ph[:],
                                     AF.Relu, bias=be1_c[:])

            for t in range(NT if phases >= 0.9 else 0):
                p0 = PSA.tile([128, HID], f32, tag="p0")
                nc.tensor.matmul(p0[:], xT[:, t * 128:(t + 1) * 128], w_in[:],
                                 start=True, stop=False)
                nc.tensor.matmul(p0[:], ones_r, b_in_r, start=False, stop=True)
                nc.scalar.activation(h_a[:, t, :], p0[:], AF.Relu)
            nc.sync.dma_start(node_rows_ap(local_out, NT, HID), h_a[:])
            psA_cm.__exit__(None, None, None)

            # ===== phase 2: Wedge build =====
            psB_cm = tc.tile_pool(name="psB", bufs=2, space="PSUM")
            PSB = psB_cm.__enter__()
            for t in range(ET if phases >= 2 else 0):
                wsb = L.tile([128, 4096], bf16, tag="wt")
                for half in range(2):
                    pw = PSB.tile([128, 2048], f32, tag="pw")
                    for j in range(4):
                        c = half * 2048 + j * 512
                        nc.tensor.matmul(
                            pw[:, j * 512:(j + 1) * 512], ones_r,
                            be2_r[:, c:c + 512],
                            start=True, stop=False)
                        nc.tensor.matmul(
                            pw[:, j * 512:(j + 1) * 512],
                            h2T[:, t * 128:(t + 1) * 128],
                            we2[:, c:c + 512],
                            start=False, stop=True)
                    if half == 0:
                        nc.scalar.copy(wsb[:, 0:2048], pw[:])
                    else:
                        nc.vector.tensor_copy(wsb[:, 2048:4096], pw[:])
                nc.sync.dma_start(wedge_hbm[t], wsb[:])
            psB_cm.__exit__(None, None, None)
            B_cm.__exit__(None, None, None)

            # ===== iterations =====
            if phases >= 2.2:
                idx2 = load(P, idx_in, (128, 2 * IDXC), i16)
                gidx = idx2[:, 0:IDXC]
                sidx = idx2[:, IDXC:2 * IDXC]
                srel = load(P, srel_in, (128, ET, 128), bf16)
                bconv = load(W, bconv_in, (128, HID), f32)
                h_b = P.tile([128, NT, HID], f32)
                msgred = P.tile([128, ET, HID], f32)
                gat = P.tile([128, ET, HID], f32)
                agg = P.tile([128, NT, HID], f32)
                agg_init = P.tile([128, AGG_T, HID], f32)
                rep_in = bass.AP(tensor=bconv[:].tensor,
                                 offset=bconv[:].offset,
                                 ap=[bconv[:].ap[0], [0, AGG_T], [1, HID]])
                nc.vector.tensor_copy(agg_init[:], rep_in)

            psS_cm = tc.tile_pool(name="psS", bufs=2, space="PSUM")
            PSS = psS_cm.__enter__()
            psG_cm = tc.tile_pool(name="psG", bufs=4, space="PSUM")
            PSG = psG_cm.__enter__()

            h_cur, h_nxt = h_a, (h_b if phases >= 2.2 else h_a)
            for it in range(NUM_NN_ITER if phases >= 3 else
                            (1 if phases >= 2.2 else 0)):
                nc.gpsimd.collective_compute(
                    "AllGather", ALU.bypass,
                    replica_groups=[list(range(NCORE))],
                    ins=[local_out[:]], outs=[table[:]])
                if phases >= 2.4:
                    for k in range(ET // 2):
                        nc.gpsimd.dma_gather(
                            out_ap=gat[:, k * 2:(k + 1) * 2, :],
                            in_ap=table[:],
                            idxs_ap=idx2[:, k * 16:(k + 1) * 16],
                            num_idxs=256, num_idxs_reg=256,
                            elem_size=HID)
                    nc.sync.dma_start(node_rows_ap(agg_hbm, AGG_T, HID),
                                       agg_init[:])
                for t in range(ET if phases >= 2.6 else 0):
                    wtile = L.tile([128, 4096], bf16, tag="wt")
                    nc.sync.dma_start(wtile[:], wedge_hbm[t])
                    dup2 = S.tile([128, 128], bf16, tag="dup2")
                    g_sl = gat[:, t, :]
                    nc.vector.tensor_copy(
                        dup2[:].rearrange("p (i w) -> p i w", w=2),
                        bass.AP(tensor=g_sl.tensor, offset=g_sl.offset,
                                ap=[g_sl.ap[0], [1, HID], [0, 2]]))
                    tmp = L.tile([128, 4096], bf16, tag="tmp")
                    d_ap = dup2[:]
                    nc.vector.tensor_tensor(
                        tmp[:].rearrange("p (i o2 w) -> p i o2 w", o2=32, w=2),
                        wtile[:].rearrange("p (i o2 w) -> p i o2 w", o2=32, w=2),
                        bass.AP(tensor=d_ap.tensor, offset=d_ap.offset,
                                ap=[d_ap.ap[0], [2, HID], [0, 32], [1, 2]]),
                        op=ALU.mult)
                    pss = PSS.tile([128, HID], f32, tag="pss")
                    o_ap = pss[:]
                    rv = bass.AP(tensor=o_ap.tensor, offset=o_ap.offset,
                                 ap=[o_ap.ap[0], [0, 8], [1, HID]])
                    for q in range(8):
                        nc.tensor.matmul(
                            rv, srel[:, t, :],
                            tmp[:, q * 512:(q + 1) * 512],
                            start=(q == 0), stop=(q == 7))
                    nc.scalar.copy(msgred[:, t, :], pss[:])

                tc.strict_bb_all_engine_barrier()
                if phases >= 2.8:
                    for k in range(ET // 2):
                        nc.gpsimd.dma_scatter_add(
                            out_ap=agg_hbm[:],
                            in_ap=msgred[:, k * 2:(k + 1) * 2, :],
                            idxs_ap=idx2[:, IDXC + k * 16:IDXC + (k + 1) * 16],
                            num_idxs=256, num_idxs_reg=256,
                            elem_size=HID)
                if phases >= 3:
                    nc.sync.dma_start(agg[:], node_rows_ap(agg_hbm, NT, HID))

                for t in range(NT if phases >= 3 else 0):
                    pth = PSG.tile([128, 3 * HID], f32, tag="gru")
                    nc.tensor.transpose(pth[0:64, 0:128], h_cur[:, t, :], ident)
                    hTt = SC.tile([64, 128], f32, tag="sc")
                    nc.vector.tensor_copy(hTt[:], pth[0:64, 0:128])
                    pg = PSG.tile([128, 3 * HID], f32, tag="gru")
                    nc.tensor.matmul(pg[:, 0:HID], hTt[:], w_root,
                                     start=True, stop=True)
                    pre = SC.tile([128, HID], f32, tag="sc")
                    nc.vector.tensor_add(pre[:], pg[:, 0:HID], agg[:, t, :])
                    m = SC.tile([128, HID], f32, tag="sc")
                    nc.scalar.activation(m[:], pre[:], AF.Relu)
                    pmt = PSG.tile([128, 3 * HID], f32, tag="gru")
                    nc.tensor.transpose(pmt[0:64, 0:128], m[:], ident)
                    mT = SC.tile([64, 128], f32, tag="sc")
                    nc.vector.tensor_copy(mT[:], pmt[0:64, 0:128])

                    pgi = PSG.tile([128, 3 * HID], f32, tag="gru")
                    nc.tensor.matmul(pgi[:], mT[:], w_ihT,
                                     start=True, stop=False)
                    nc.tensor.matmul(pgi[:], ones_r, b_ih_r,
                                     start=False, stop=True)
                    pgh = PSG.tile([128, 3 * HID], f32, tag="gru")
                    nc.tensor.matmul(pgh[:], hTt[:], w_hhT,
                                     start=True, stop=False)
                    nc.tensor.matmul(pgh[:], ones_r, b_hh_r,
                                     start=False, stop=True)
                    gi = SC.tile([128, 3 * HID], f32, tag="sc")
                    nc.vector.tensor_copy(gi[:], pgi[:])

                    trz = SC.tile([128, 2 * HID], f32, tag="sc")
                    nc.vector.tensor_add(trz[:], gi[:, 0:128], pgh[:, 0:128])
                    rz = SC.tile([128, 2 * HID], f32, tag="sc")
                    nc.scalar.activation(rz[:], trz[:], AF.Sigmoid)
                    tn1 = SC.tile([128, HID], f32, tag="sc")
                    nc.vector.tensor_mul(tn1[:], rz[:, 0:HID], pgh[:, 128:192])
                    tn2 = SC.tile([128, HID], f32, tag="sc")
                    nc.vector.tensor_add(tn2[:], tn1[:], gi[:, 128:192])
                    nn_ = SC.tile([128, HID], f32, tag="sc")
                    nc.scalar.activation(nn_[:], tn2[:], AF.Tanh)
                    th1 = SC.tile([128, HID], f32, tag="sc")
                    nc.vector.tensor_sub(th1[:], h_cur[:, t, :], nn_[:])
                    th2 = SC.tile([128, HID], f32, tag="sc")
                    nc.vector.tensor_mul(th2[:], rz[:, HID:128], th1[:])
                    nc.vector.tensor_add(h_nxt[:, t, :], nn_[:], th2[:])

                if it == 0 and phases >= 3:
                    nc.sync.dma_start(node_rows_ap(dbg_out, NT, HID), h_nxt[:])
                if phases >= 3 and it < NUM_NN_ITER - 1:
                    nc.sync.dma_start(node_rows_ap(local_out, NT, HID),
                                      h_nxt[:])
                h_cur, h_nxt = h_nxt, h_cur
            psG_cm.__exit__(None, None, None)
            psS_cm.__exit__(None, None, None)

            # ===== tail =====
            psT_cm = tc.tile_pool(name="psT", bufs=2, space="PSUM")
            PST = psT_cm.__enter__()
            psR_cm = tc.tile_pool(name="psR", bufs=1, space="PSUM")
            PSR = psR_cm.__enter__()
            if phases >= 4:
                xloc = load(P, xloc_in, (128, NT, NUM_ATOMS), f32)
                sbro = load(P, sb_in, (128, NT, RO_W), f32)
                wp = load(W, wp_in, (RO_W, FEAT), f32)
                bp = load(W, bp_in, (RO_W, 1), f32)
                pro = PSR.tile([RO_W, FEAT], f32, tag="pro")
                pcnt = PSR.tile([RO_W, 1], f32, tag="pcnt")
            for t in range(NT if phases >= 4 else 0):
                p2h = PST.tile([128, 128], f32, tag="tl")
                nc.tensor.transpose(p2h[0:64, 0:128], h_cur[:, t, :], ident)
                hTt2 = SC.tile([64, 128], f32, tag="sc")
                nc.vector.tensor_copy(hTt2[:], p2h[0:64, 0:128])
                po1 = PST.tile([128, 128], f32, tag="tl")
                nc.tensor.matmul(po1[:, 0:HID], hTt2[:], wo1,
                                 start=True, stop=False)
                nc.tensor.matmul(po1[:, 0:HID], ones_r, bo1_r,
                                 start=False, stop=True)
                o1 = SC.tile([128, HID], f32, tag="sc")
                nc.scalar.activation(o1[:], po1[:, 0:HID], AF.Relu)
                p1t = PST.tile([128, 128], f32, tag="tl")
                nc.tensor.transpose(p1t[0:64, 0:128], o1[:], ident)
                o1T = SC.tile([64, 128], f32, tag="sc")
                nc.vector.tensor_copy(o1T[:], p1t[0:64, 0:128])
                po2 = PST.tile([128, 128], f32, tag="tl")
                nc.tensor.matmul(po2[:, 0:EMB], o1T[:], wo2,
                                 start=True, stop=False)
                nc.tensor.matmul(po2[:, 0:EMB], ones_r, bo2_r,
                                 start=False, stop=True)
                feat = SC.tile([128, FEAT], f32, tag="sc")
                nc.vector.tensor_copy(feat[:, 0:EMB], po2[:, 0:EMB])
                nc.vector.tensor_copy(feat[:, EMB:FEAT], xloc[:, t, :])
                fsq = SC.tile([128, FEAT], f32, tag="sc")
                nc.scalar.square(fsq[:], feat[:])
                ss = SC.tile([128, 1], f32, tag="sc")
                nc.vector.tensor_reduce(ss[:], fsq[:], mybir.AxisListType.X,
                                        ALU.add)
                nrm = SC.tile([128, 1], f32, tag="sc")
                nc.scalar.sqrt(nrm[:], ss[:])
                nrm2 = SC.tile([128, 1], f32, tag="sc")
                nc.vector.tensor_scalar_max(nrm2[:], nrm[:], 1e-12)
                rinv = SC.tile([128, 1], f32, tag="sc")
                nc.vector.reciprocal(rinv[:], nrm2[:])
                featn = SC.tile([128, FEAT], f32, tag="sc")
                nc.vector.tensor_scalar_mul(featn[:], feat[:], rinv[:])
                nc.sync.dma_start(
                    bass.AP(tensor=feat_out, offset=t * 128 * FEAT,
                            ap=[[FEAT, 128], [1, FEAT]]),
                    featn[:])
                nc.tensor.matmul(pro[:], sbro[:, t, :], featn[:],
                                 start=(t == 0), stop=(t == NT - 1),
                                 skip_group_check=True)
                nc.tensor.matmul(pcnt[:], sbro[:, t, :], ones_c,
                                 start=(t == 0), stop=(t == NT - 1),
                                 skip_group_check=True)

            if phases >= 4:
                cnt = SC.tile([RO_W, 1], f32, tag="sc")
                nc.vector.tensor_scalar_max(cnt[:], pcnt[:], 1.0)
                rc = SC.tile([RO_W, 1], f32, tag="sc")
                nc.vector.reciprocal(rc[:], cnt[:])
                rm = SC.tile([RO_W, FEAT], f32, tag="sc")
                nc.vector.tensor_scalar_mul(rm[:], pro[:], rc[:])
                tt = SC.tile([RO_W, FEAT], f32, tag="sc")
                nc.vector.tensor_mul(tt[:], rm[:], wp[:])
                s1 = SC.tile([RO_W, 1], f32, tag="sc")
                nc.vector.tensor_reduce(s1[:], tt[:], mybir.AxisListType.X,
                                        ALU.add)
                ratio = SC.tile([RO_W, 1], f32, tag="sc")
                nc.scalar.activation(ratio[:], s1[:], AF.Sigmoid, bias=bp[:])
                nc.sync.dma_start(ratio_out[:], ratio[:])
            psR_cm.__exit__(None, None, None)
            psT_cm.__exit__(None, None, None)

    nc.compile()
    return nc


_PROGRAM_CACHE = {}
_LAST_RESULTS = None


def kernel(**inputs):
    global _LAST_RESULTS
    x = np.asarray(inputs["x"], np.float32)
    edge_index = np.asarray(inputs["edge_index"], np.int64)
    edge_attr = np.asarray(inputs["edge_attr"], np.float32)
    batch = np.asarray(inputs["batch"], np.int64)

    Wt = {k: np.asarray(v, np.float32) for k, v in inputs.items()
          if k not in ("x", "edge_index", "edge_attr", "batch")}

    src, dst = edge_index[0], edge_index[1]
    node_start = np.searchsorted(batch, np.arange(0, N_GRAPHS + 1, GPC))
    core_of_node = np.minimum(batch // GPC, NCORE - 1)
    loc_id = np.arange(N_NODES) - node_start[core_of_node]
    gid = core_of_node * NPAD + loc_id

    wpack = np.concatenate(
        [Wt["W_root"], Wt["Wo1"], Wt["Wo2"],
         np.ascontiguousarray(Wt["W_ih"].T),
         np.ascontiguousarray(Wt["W_hh"].T)], axis=1).astype(np.float32)
    bpack = np.concatenate(
        [Wt["b_in"], Wt["b_ih"], Wt["b_hh"], Wt["bo1"], Wt["bo2"]]
    ).reshape(1, -1).astype(np.float32)

    in_maps = []
    per_core = []
    for c in range(NCORE):
        ns, ne = int(node_start[c]), int(node_start[c + 1])
        nlc = ne - ns
        assert nlc <= NPAD, f"core {c} has {nlc} nodes > {NPAD}"

        eidx = np.nonzero(core_of_node[dst] == c)[0]
        d_loc = (dst[eidx] - ns).astype(np.int64)
        order = np.argsort(d_loc, kind="stable")
        eidx = eidx[order]
        d_loc = d_loc[order]

        uniq, counts = np.unique(d_loc, return_counts=True)
        assert len(uniq) == 0 or counts.max() <= 128
        tiles = []
        cur = []
        pos = 0
        for u, cnt in zip(uniq, counts):
            if len(cur) + cnt > 128:
                cur.extend([-1] * (128 - len(cur)))
                tiles.append(cur)
                cur = []
            cur.extend(range(pos, pos + cnt))
            pos += cnt
        if cur:
            cur.extend([-1] * (128 - len(cur)))
            tiles.append(cur)
        assert len(tiles) <= ET, f"core {c}: {len(tiles)} tiles > {ET}"
        while len(tiles) < ET:
            tiles.append([-1] * 128)

        g_ids = np.zeros(EPAD, np.int64)
        s_ids = np.full(EPAD, NPAD, np.int64)     # default -> trash row
        srel = np.zeros((128, ET, 128), np.float32)
        ea_perm = np.zeros((EPAD, 4), np.float32)
        for t, tl in enumerate(tiles):
            slot_of = {}
            for r, p in enumerate(tl):
                if p < 0:
                    continue
                e = eidx[p]
                g_ids[t * 128 + r] = gid[src[e]]
                ea_perm[t * 128 + r] = edge_attr[e]
                d = d_loc[p]
                if d not in slot_of:
                    slot_of[d] = len(slot_of)
                    s_ids[t * 128 + slot_of[d]] = d
                srel[r, t, slot_of[d]] = 1.0

        x_pad = np.zeros((NPAD, NUM_ATOMS), np.float32)
        x_pad[:nlc] = x[ns:ne]
        sbro = np.zeros((NPAD, RO_W), np.float32)
        col = np.full(NPAD, GPC, np.int64)
        col[:nlc] = batch[ns:ne] - c * GPC
        sbro[np.arange(NPAD), col] = 1.0

        in_maps.append({
            "eaT": np.ascontiguousarray(ea_perm.T),
            "idx2": np.concatenate([_wrap_idx(g_ids), _wrap_idx(s_ids)],
                                   axis=1),
            "srel": srel.astype(ml_dtypes.bfloat16),
            "xT": np.ascontiguousarray(x_pad.T),
            "xloc": _tile_major(x_pad, NT),
            "sbro": _tile_major(sbro, NT),
            "w_in": Wt["W_in"],
            "we1": Wt["We1"],
            "be1_c": Wt["be1"].reshape(-1, 1),
            "we2": Wt["We2"].astype(ml_dtypes.bfloat16),
            "be2_r": Wt["be2"].reshape(1, -1),
            "wpack": wpack,
            "bpack": bpack,
            "bconv_rep": np.tile(Wt["b_conv"].reshape(1, -1), (128, 1)),
            "wp_rep": np.tile(Wt["Wp"].reshape(1, -1), (RO_W, 1)),
            "bp_rep": np.tile(Wt["bp"].reshape(1, 1), (RO_W, 1)),
        })
        per_core.append((ns, ne, nlc))

    if "prog" not in _PROGRAM_CACHE:
        _PROGRAM_CACHE["prog"] = _build_program()
    nc = _PROGRAM_CACHE["prog"]

    import time as _time
    _t0 = _time.time()
    res = run_bass_kernel_spmd(nc, in_maps, list(range(NCORE))).results
    globals()["_LAST_RUN_S"] = _time.time() - _t0
    _LAST_RESULTS = res

    feat = np.empty((N_NODES, FEAT), np.float32)
    ratio = np.empty((N_GRAPHS, 1), np.float32)
    for c, (ns, ne, nlc) in enumerate(per_core):
        feat[ns:ne] = res[c]["feat_out"][:nlc]
        ratio[c * GPC:(c + 1) * GPC] = res[c]["ratio_out"][:GPC]
    return feat, ratio


# revision 29
# speedup vs baseline: 1.0553x; 1.0553x over previous
"""CGNet (NNConv + GRU message passing) Trainium2 kernel, 8-way graph-parallel.

Sharding: graphs (and their nodes via the sorted batch vector) are
partitioned across 8 cores; edges live on the core owning their dst node.
Per-edge weight blocks Wedge = edgeMLP(edge_attr) are built once on device
(TensorE, bf16) and staged in HBM. Each iteration: AllGather the node
table, dma_gather out[src] rows, multiply Wedge tiles by the replicated
source vectors on VectorE, then reduce over the input dim AND scatter to
per-tile dst slots in one TensorE pass (one-hot stationary + PSUM revisit
accumulation), dma_scatter_add unique dst rows into an HBM agg table
pre-filled with b_conv, and run the GRU cell per node tile in fp32.
"""
import sys

sys.path.insert(0, "/opt/trn_rl_repo")

import numpy as np
import ml_dtypes

import concourse.bass as bass
import concourse.mybir as mybir
import concourse.tile as tile
from concourse import bacc
from concourse.bass_utils import run_bass_kernel_spmd
from concourse.masks import make_identity

f32 = mybir.dt.float32
bf16 = mybir.dt.bfloat16
i16 = mybir.dt.int16
AF = mybir.ActivationFunctionType
ALU = mybir.AluOpType

N_NODES = 25000
N_EDGES = 50000
NUM_ATOMS = 26
HID = 64
EMB = 64
N_GRAPHS = 64
NUM_NN_ITER = 3
NCORE = 8
GPC = N_GRAPHS // NCORE

NT = 28
NPAD = NT * 128              # 3584 padded nodes per core
ET = 52
EPAD = ET * 128              # 6656 padded edges per core
NTAB = NPAD * NCORE
AGG_T = NT + 1               # +1 trash tile for pad slots
AGG_ROWS = AGG_T * 128
IDXC = EPAD // 16
RO_W = 16
FEAT = EMB + NUM_ATOMS


def _wrap_idx(idx):
    a = np.asarray(idx, dtype=np.int64)
    assert a.shape[0] == EPAD
    w = a.reshape(IDXC, 16).T.astype(np.int16)
    return np.tile(w, (8, 1))


def _tile_major(rows, ntiles):
    d = rows.shape[1]
    return np.ascontiguousarray(rows.reshape(ntiles, 128, d).transpose(1, 0, 2))


def _build_program(phases=4):
    nc = bacc.Bacc("TRN2", target_bir_lowering=False, debug=False,
                   num_devices=NCORE)

    def par(name, shape, dtype, out=False):
        return nc.declare_dram_parameter(name, list(shape), dtype, isOutput=out)

    eaT_in = par("eaT", (4, EPAD), f32)
    idx_in = par("idx2", (128, 2 * IDXC), i16)     # [gather | scatter]
    srel_in = par("srel", (128, ET, 128), bf16)
    xT_in = par("xT", (NUM_ATOMS, NPAD), f32)
    xloc_in = par("xloc", (128, NT, NUM_ATOMS), f32)
    sb_in = par("sbro", (128, NT, RO_W), f32)
    win_in = par("w_in", (NUM_ATOMS, HID), f32)
    we1_in = par("we1", (4, 128), f32)
    be1_in = par("be1_c", (128, 1), f32)
    we2_in = par("we2", (128, 4096), bf16)
    be2_in = par("be2_r", (1, 4096), f32)
    wpack_in = par("wpack", (HID, 576), f32)       # [Wroot|Wo1|Wo2|WihT|WhhT]
    bpack_in = par("bpack", (1, 576), f32)         # [b_in|b_ih|b_hh|bo1|bo2]
    bconv_in = par("bconv_rep", (128, HID), f32)
    wp_in = par("wp_rep", (RO_W, FEAT), f32)
    bp_in = par("bp_rep", (RO_W, 1), f32)

    feat_out = par("feat_out", (NPAD, FEAT), f32, out=True)
    ratio_out = par("ratio_out", (RO_W, 1), f32, out=True)
    dbg_out = par("dbg_h", (NPAD, HID), f32, out=True)

    wedge_hbm = nc.dram_tensor("wedge_hbm", [ET, 128, 4096], bf16)
    local_out = nc.dram_tensor("local_out", [NPAD, HID], f32)
    table = nc.dram_tensor("table", [NTAB, HID], f32, addr_space="Shared")
    agg_hbm = nc.dram_tensor("agg_hbm", [AGG_ROWS, HID], f32)

    def node_rows_ap(dram, ntiles, d):
        return bass.AP(tensor=dram, offset=0,
                       ap=[[d, 128], [128 * d, ntiles], [1, d]])

    with tile.TileContext(nc) as tc:
        with (
            tc.tile_pool(name="persist", bufs=1) as P,
            tc.tile_pool(name="wts", bufs=1) as W,
            tc.tile_pool(name="loop", bufs=2) as L,
            tc.tile_pool(name="sm2", bufs=2) as S,
            tc.tile_pool(name="sc8", bufs=8) as SC,
        ):
            def load(pool, param, shape, dtype):
                t = pool.tile(list(shape), dtype, tag=param.name)
                nc.sync.dma_start(t[:], param[:])
                return t

            B_cm = tc.tile_pool(name="build", bufs=1)
            B = B_cm.__enter__()
            xT = load(B, xT_in, (NUM_ATOMS, NPAD), f32)
            w_in = load(B, win_in, (NUM_ATOMS, HID), f32)
            we1 = load(B, we1_in, (4, 128), f32)
            be1_c = load(B, be1_in, (128, 1), f32)
            we2 = load(B, we2_in, (128, 4096), bf16)
            be2_r = load(B, be2_in, (1, 4096), f32)
            wpack = load(W, wpack_in, (HID, 576), f32)
            bpack = load(W, bpack_in, (1, 576), f32)
            w_root = wpack[:, 0:64]
            wo1 = wpack[:, 64:128]
            wo2 = wpack[:, 128:192]
            w_ihT = wpack[:, 192:384]
            w_hhT = wpack[:, 384:576]
            b_in_r = bpack[:, 0:64]
            b_ih_r = bpack[:, 64:256]
            b_hh_r = bpack[:, 256:448]
            bo1_r = bpack[:, 448:512]
            bo2_r = bpack[:, 512:576]

            # constants: [0:128]=identity, col 128 = ones col, row0[132:260] = ones row
            bigc = P.tile([128, 260], f32)
            nc.gpsimd.memset(bigc[:], 0.0)
            make_identity(nc, bigc[:, 0:128], nomemset=True)
            nc.gpsimd.memset(bigc[:, 128:129], 1.0)
            nc.gpsimd.memset(bigc[0:1, 132:260], 1.0)
            ident = bigc[:, 0:128]
            ones_c = bigc[:, 128:129]
            ones_r = bigc[0:1, 132:260]

            h_a = P.tile([128, NT, HID], f32)

            # ===== phase 1: h2T and initial node state =====
            psA_cm = tc.tile_pool(name="psA", bufs=2, space="PSUM")
            PSA = psA_cm.__enter__()
            h2T = B.tile([128, EPAD], bf16)
            for j in range(EPAD // 512 if phases >= 0.7 else 0):
                eas = B.tile([4, 512], f32, tag="eas")
                nc.sync.dma_start(eas[:], eaT_in[:, j * 512:(j + 1) * 512])
                ph = PSA.tile([128, 512], f32, tag="ph2")
                nc.tensor.matmul(ph[:], we1[:], eas[:], start=True, stop=True)
                nc.scalar.activation(h2T[:, j * 512:(j + 1) * 512], ph[:],
                                     AF.Relu, bias=be1_c[:])

            for t in range(NT if phases >= 0.9 else 0):
                p0 = PSA.tile([128, HID], f32, tag="p0")
                nc.tensor.matmul(p0[:], xT[:, t * 128:(t + 1) * 128], w_in[:],
                                 start=True, stop=False)
                nc.tensor.matmul(p0[:], ones_r, b_in_r, start=False, stop=True)
                nc.scalar.activation(h_a[:, t, :], p0[:], AF.Relu)
            nc.sync.dma_start(node_rows_ap(local_out, NT, HID), h_a[:])
            psA_cm.__exit__(None, None, None)

            # ===== phase 2: Wedge build =====
            psB_cm = tc.tile_pool(name="psB", bufs=2, space="PSUM")
            PSB = psB_cm.__enter__()
            for t in range(ET if phases >= 2 else 0):
                wsb = L.tile([128, 4096], bf16, tag="wt")
                for half in range(2):
                    pw = PSB.tile([128, 2048], f32, tag="pw")
                    for j in range(4):
                        c = half * 2048 + j * 512
                        nc.tensor.matmul(
                            pw[:, j * 512:(j + 1) * 512], ones_r,
                            be2_r[:, c:c + 512],
                            start=True, stop=False)
                        nc.tensor.matmul(
                            pw[:, j * 512:(j + 1) * 512],
                            h2T[:, t * 128:(t + 1) * 128],
                            we2[:, c:c + 512],
                            start=False, stop=True)
                    if half == 0:
                        nc.scalar.copy(wsb[:, 0:2048], pw[:])
                    else:
                        nc.vector.tensor_copy(wsb[:, 2048:4096], pw[:])
                nc.sync.dma_start(wedge_hbm[t], wsb[:])
            psB_cm.__exit__(None, None, None)
            B_cm.__exit__(None, None, None)

            # ===== iterations =====
            if phases >= 2.2:
                idx2 = load(P, idx_in, (128, 2 * IDXC), i16)
                gidx = idx2[:, 0:IDXC]
                sidx = idx2[:, IDXC:2 * IDXC]
                srel = load(P, srel_in, (128, ET, 128), bf16)
                bconv = load(W, bconv_in, (128, HID), f32)
                h_b = P.tile([128, NT, HID], f32)
                msgred = P.tile([128, ET, HID], f32)
                gat = P.tile([128, ET, HID], f32)
                agg = P.tile([128, NT, HID], f32)
                agg_init = P.tile([128, AGG_T, HID], f32)
                rep_in = bass.AP(tensor=bconv[:].tensor,
                                 offset=bconv[:].offset,
                                 ap=[bconv[:].ap[0], [0, AGG_T], [1, HID]])
                nc.vector.tensor_copy(agg_init[:], rep_in)

            psS_cm = tc.tile_pool(name="psS", bufs=2, space="PSUM")
            PSS = psS_cm.__enter__()
            psG_cm = tc.tile_pool(name="psG", bufs=4, space="PSUM")
            PSG = psG_cm.__enter__()

            h_cur, h_nxt = h_a, (h_b if phases >= 2.2 else h_a)
            for it in range(NUM_NN_ITER if phases >= 3 else
                            (1 if phases >= 2.2 else 0)):
                nc.gpsimd.collective_compute(
                    "AllGather", ALU.bypass,
                    replica_groups=[list(range(NCORE))],
                    ins=[local_out[:]], outs=[table[:]])
                if phases >= 2.4:
                    for k in range(ET // 4):
                        nc.gpsimd.dma_gather(
                            out_ap=gat[:, k * 4:(k + 1) * 4, :],
                            in_ap=table[:],
                            idxs_ap=idx2[:, k * 32:(k + 1) * 32],
                            num_idxs=512, num_idxs_reg=512,
                            elem_size=HID)
                    nc.sync.dma_start(node_rows_ap(agg_hbm, AGG_T, HID),
                                       agg_init[:])
                for t in range(ET if phases >= 2.6 else 0):
                    wtile = L.tile([128, 4096], bf16, tag="wt")
                    nc.sync.dma_start(wtile[:], wedge_hbm[t])
                    dup2 = S.tile([128, 128], bf16, tag="dup2")
                    g_sl = gat[:, t, :]
                    nc.vector.tensor_copy(
                        dup2[:].rearrange("p (i w) -> p i w", w=2),
                        bass.AP(tensor=g_sl.tensor, offset=g_sl.offset,
                                ap=[g_sl.ap[0], [1, HID], [0, 2]]))
                    tmp = L.tile([128, 4096], bf16, tag="tmp")
                    d_ap = dup2[:]
                    nc.vector.tensor_tensor(
                        tmp[:].rearrange("p (i o2 w) -> p i o2 w", o2=32, w=2),
                        wtile[:].rearrange("p (i o2 w) -> p i o2 w", o2=32, w=2),
                        bass.AP(tensor=d_ap.tensor, offset=d_ap.offset,
                                ap=[d_ap.ap[0], [2, HID], [0, 32], [1, 2]]),
                        op=ALU.mult)
                    pss = PSS.tile([128, HID], f32, tag="pss")
                    o_ap = pss[:]
                    rv = bass.AP(tensor=o_ap.tensor, offset=o_ap.offset,
                                 ap=[o_ap.ap[0], [0, 8], [1, HID]])
                    for q in range(8):
                        nc.tensor.matmul(
                            rv, srel[:, t, :],
                            tmp[:, q * 512:(q + 1) * 512],
                            start=(q == 0), stop=(q == 7))
                    nc.scalar.copy(msgred[:, t, :], pss[:])

                tc.strict_bb_all_engine_barrier()
                if phases >= 2.8:
                    for k in range(ET // 4):
                        nc.gpsimd.dma_scatter_add(
                            out_ap=agg_hbm[:],
                            in_ap=msgred[:, k * 4:(k + 1) * 4, :],
                            idxs_ap=idx2[:, IDXC + k * 32:IDXC + (k + 1) * 32],
                            num_idxs=512, num_idxs_reg=512,
                            elem_size=HID)
                if phases >= 3:
                    nc.sync.dma_start(agg[:], node_rows_ap(agg_hbm, NT, HID))

                for t in range(NT if phases >= 3 else 0):
                    pth = PSG.tile([128, 3 * HID], f32, tag="gru")
                    nc.tensor.transpose(pth[0:64, 0:128], h_cur[:, t, :], ident)
                    hTt = SC.tile([64, 128], f32, tag="sc")
                    nc.vector.tensor_copy(hTt[:], pth[0:64, 0:128])
                    pg = PSG.tile([128, 3 * HID], f32, tag="gru")
                    nc.tensor.matmul(pg[:, 0:HID], hTt[:], w_root,
                                     start=True, stop=True)
                    pre = SC.tile([128, HID], f32, tag="sc")
                    nc.vector.tensor_add(pre[:], pg[:, 0:HID], agg[:, t, :])
                    m = SC.tile([128, HID], f32, tag="sc")
                    nc.scalar.activation(m[:], pre[:], AF.Relu)
                    pmt = PSG.tile([128, 3 * HID], f32, tag="gru")
                    nc.tensor.transpose(pmt[0:64, 0:128], m[:], ident)
                    mT = SC.tile([64, 128], f32, tag="sc")
                    nc.vector.tensor_copy(mT[:], pmt[0:64, 0:128])

                    pgi = PSG.tile([128, 3 * HID], f32, tag="gru")
                    nc.tensor.matmul(pgi[:], mT[:], w_ihT,
                                     start=True, stop=False)
                    nc.tensor.matmul(pgi[:], ones_r, b_ih_r,
                                     start=False, stop=True)
                    pgh = PSG.tile([128, 3 * HID], f32, tag="gru")
                    nc.tensor.matmul(pgh[:], hTt[:], w_hhT,
                                     start=True, stop=False)
                    nc.tensor.matmul(pgh[:], ones_r, b_hh_r,
                                     start=False, stop=True)
                    gi = SC.tile([128, 3 * HID], f32, tag="sc")
                    nc.vector.tensor_copy(gi[:], pgi[:])

                    trz = SC.tile([128, 2 * HID], f32, tag="sc")
                    nc.vector.tensor_add(trz[:], gi[:, 0:128], pgh[:, 0:128])
                    rz = SC.tile([128, 2 * HID], f32, tag="sc")
                    nc.scalar.activation(rz[:], trz[:], AF.Sigmoid)
                    tn1 = SC.tile([128, HID], f32, tag="sc")
                    nc.vector.tensor_mul(tn1[:], rz[:, 0:HID], pgh[:, 128:192])
                    tn2 = SC.tile([128, HID], f32, tag="sc")
                    nc.vector.tensor_add(tn2[:], tn1[:], gi[:, 128:192])
                    nn_ = SC.tile([128, HID], f32, tag="sc")
                    nc.scalar.activation(nn_[:], tn2[:], AF.Tanh)
                    th1 = SC.tile([128, HID], f32, tag="sc")
                    nc.vector.tensor_sub(th1[:], h_cur[:, t, :], nn_[:])
                    th2 = SC.tile([128, HID], f32, tag="sc")
                    nc.vector.tensor_mul(th2[:], rz[:, HID:128], th1[:])
                    nc.vector.tensor_add(h_nxt[:, t, :], nn_[:], th2[:])

                if it == 0 and phases >= 3:
                    nc.sync.dma_start(node_rows_ap(dbg_out, NT, HID), h_nxt[:])
                if phases >= 3 and it < NUM_NN_ITER - 1:
                    nc.sync.dma_start(node_rows_ap(local_out, NT, HID),
                                      h_nxt[:])
                h_cur, h_nxt = h_nxt, h_cur
            psG_cm.__exit__(None, None, None)
            psS_cm.__exit__(None, None, None)

            # ===== tail =====
            psT_cm = tc.tile_pool(name="psT", bufs=2, space="PSUM")
            PST = psT_cm.__enter__()
            psR_cm = tc.tile_pool(name="psR", bufs=1, space="PSUM")
            PSR = psR_cm.__enter__()
            if phases >= 4:
                xloc = load(P, xloc_in, (128, NT, NUM_ATOMS), f32)
                sbro = load(P, sb_in, (128, NT, RO_W), f32)
                wp = load(W, wp_in, (RO_W, FEAT), f32)
                bp = load(W, bp_in, (RO_W, 1), f32)
                pro = PSR.tile([RO_W, FEAT], f32, tag="pro")
                pcnt = PSR.tile([RO_W, 1], f32, tag="pcnt")
            for t in range(NT if phases >= 4 else 0):
                p2h = PST.tile([128, 128], f32, tag="tl")
                nc.tensor.transpose(p2h[0:64, 0:128], h_cur[:, t, :], ident)
                hTt2 = SC.tile([64, 128], f32, tag="sc")
                nc.vector.tensor_copy(hTt2[:], p2h[0:64, 0:128])
                po1 = PST.tile([128, 128], f32, tag="tl")
                nc.tensor.matmul(po1[:, 0:HID], hTt2[:], wo1,
                                 start=True, stop=False)
                nc.tensor.matmul(po1[:, 0:HID], ones_r, bo1_r,
                                 start=False, stop=True)
                o1 = SC.tile([128, HID], f32, tag="sc")
                nc.scalar.activation(o1[:], po1[:, 0:HID], AF.Relu)
                p1t = PST.tile([128, 128], f32, tag="tl")
                nc.tensor.transpose(p1t[0:64, 0:128], o1[:], ident)
                o1T = SC.tile([64, 128], f32, tag="sc")
                nc.vector.tensor_copy(o1T[:], p1t[0:64, 0:128])
                po2 = PST.tile([128, 128], f32, tag="tl")
                nc.tensor.matmul(po2[:, 0:EMB], o1T[:], wo2,
                                 start=True, stop=False)
                nc.tensor.matmul(po2[:, 0:EMB], ones_r, bo2_r,
                                 start=False, stop=True)
                feat = SC.tile([128, FEAT], f32, tag="sc")
                nc.vector.tensor_copy(feat[:, 0:EMB], po2[:, 0:EMB])
                nc.vector.tensor_copy(feat[:, EMB:FEAT], xloc[:, t, :])
                fsq = SC.tile([128, FEAT], f32, tag="sc")
                nc.scalar.square(fsq[:], feat[:])
                ss = SC.tile([128, 1], f32, tag="sc")
                nc.vector.tensor_reduce(ss[:], fsq[:], mybir.AxisListType.X,
                                        ALU.add)
                nrm = SC.tile([128, 1], f32, tag="sc")
                nc.scalar.sqrt(nrm[:], ss[:])
                nrm2 = SC.tile([128, 1], f32, tag="sc")
                nc.vector.tensor_scalar_max(nrm2[:], nrm[:], 1e-12)
                rinv = SC.tile([128, 1], f32, tag="sc")
                nc.vector.reciprocal(rinv[:], nrm2[:])
                featn = SC.tile([128, FEAT], f32, tag="sc")
                nc.vector.tensor_scalar_mul(featn[:], feat[:], rinv[:])
                nc.sync.dma_start(
                    bass.AP(tensor=feat_out, offset=t * 128 * FEAT,
                            ap=[[FEAT, 128], [1, FEAT]]),
                    featn[:])
                nc.tensor.matmul(pro[:], sbro[:, t, :], featn[:],
                                 start=(t == 0), stop=(t == NT - 1),
                                 skip_group_check=True)
                nc.tensor.matmul(pcnt[:], sbro[:, t, :], ones_c,
                                 start=(t == 0), stop=(t == NT - 1),
                                 skip_group_check=True)

            if phases >= 4:
                cnt = SC.tile([RO_W, 1], f32, tag="sc")
                nc.vector.tensor_scalar_max(cnt[:], pcnt[:], 1.0)
                rc = SC.tile([RO_W, 1], f32, tag="sc")
                nc.vector.reciprocal(rc[:], cnt[:])
                rm = SC.tile([RO_W, FEAT], f32, tag="sc")
                nc.vector.tensor_scalar_mul(rm[:], pro[:], rc[:])
                tt = SC.tile([RO_W, FEAT], f32, tag="sc")
                nc.vector.tensor_mul(tt[:], rm[:], wp[:])
                s1 = SC.tile([RO_W, 1], f32, tag="sc")
                nc.vector.tensor_reduce(s1[:], tt[:], mybir.AxisListType.X,
                                        ALU.add)
                ratio = SC.tile([RO_W, 1], f32, tag="sc")
                nc.scalar.activation(ratio[:], s1[:], AF.Sigmoid, bias=bp[:])
                nc.sync.dma_start(ratio_out[:], ratio[:])
            psR_cm.__exit__(None, None, None)
            psT_cm.__exit__(None, None, None)

    nc.compile()
    return nc


_PROGRAM_CACHE = {}
_LAST_RESULTS = None


def kernel(**inputs):
    global _LAST_RESULTS
    x = np.asarray(inputs["x"], np.float32)
    edge_index = np.asarray(inputs["edge_index"], np.int64)
    edge_attr = np.asarray(inputs["edge_attr"], np.float32)
    batch = np.asarray(inputs["batch"], np.int64)

    Wt = {k: np.asarray(v, np.float32) for k, v in inputs.items()
          if k not in ("x", "edge_index", "edge_attr", "batch")}

    src, dst = edge_index[0], edge_index[1]
    node_start = np.searchsorted(batch, np.arange(0, N_GRAPHS + 1, GPC))
    core_of_node = np.minimum(batch // GPC, NCORE - 1)
    loc_id = np.arange(N_NODES) - node_start[core_of_node]
    gid = core_of_node * NPAD + loc_id

    wpack = np.concatenate(
        [Wt["W_root"], Wt["Wo1"], Wt["Wo2"],
         np.ascontiguousarray(Wt["W_ih"].T),
         np.ascontiguousarray(Wt["W_hh"].T)], axis=1).astype(np.float32)
    bpack = np.concatenate(
        [Wt["b_in"], Wt["b_ih"], Wt["b_hh"], Wt["bo1"], Wt["bo2"]]
    ).reshape(1, -1).astype(np.float32)

    in_maps = []
    per_core = []
    for c in range(NCORE):
        ns, ne = int(node_start[c]), int(node_start[c + 1])
        nlc = ne - ns
        assert nlc <= NPAD, f"core {c} has {nlc} nodes > {NPAD}"

        eidx = np.nonzero(core_of_node[dst] == c)[0]
        d_loc = (dst[eidx] - ns).astype(np.int64)
        order = np.argsort(d_loc, kind="stable")
        eidx = eidx[order]
        d_loc = d_loc[order]

        uniq, counts = np.unique(d_loc, return_counts=True)
        assert len(uniq) == 0 or counts.max() <= 128
        tiles = []
        cur = []
        pos = 0
        for u, cnt in zip(uniq, counts):
            if len(cur) + cnt > 128:
                cur.extend([-1] * (128 - len(cur)))
                tiles.append(cur)
                cur = []
            cur.extend(range(pos, pos + cnt))
            pos += cnt
        if cur:
            cur.extend([-1] * (128 - len(cur)))
            tiles.append(cur)
        assert len(tiles) <= ET, f"core {c}: {len(tiles)} tiles > {ET}"
        while len(tiles) < ET:
            tiles.append([-1] * 128)

        g_ids = np.zeros(EPAD, np.int64)
        s_ids = np.full(EPAD, NPAD, np.int64)     # default -> trash row
        srel = np.zeros((128, ET, 128), np.float32)
        ea_perm = np.zeros((EPAD, 4), np.float32)
        for t, tl in enumerate(tiles):
            slot_of = {}
            for r, p in enumerate(tl):
                if p < 0:
                    continue
                e = eidx[p]
                g_ids[t * 128 + r] = gid[src[e]]
                ea_perm[t * 128 + r] = edge_attr[e]
                d = d_loc[p]
                if d not in slot_of:
                    slot_of[d] = len(slot_of)
                    s_ids[t * 128 + slot_of[d]] = d
                srel[r, t, slot_of[d]] = 1.0

        x_pad = np.zeros((NPAD, NUM_ATOMS), np.float32)
        x_pad[:nlc] = x[ns:ne]
        sbro = np.zeros((NPAD, RO_W), np.float32)
        col = np.full(NPAD, GPC, np.int64)
        col[:nlc] = batch[ns:ne] - c * GPC
        sbro[np.arange(NPAD), col] = 1.0

        in_maps.append({
            "eaT": np.ascontiguousarray(ea_perm.T),
            "idx2": np.concatenate([_wrap_idx(g_ids), _wrap_idx(s_ids)],
                                   axis=1),
            "srel": srel.astype(ml_dtypes.bfloat16),
            "xT": np.ascontiguousarray(x_pad.T),
            "xloc": _tile_major(x_pad, NT),
            "sbro": _tile_major(sbro, NT),
            "w_in": Wt["W_in"],
            "we1": Wt["We1"],
            "be1_c": Wt["be1"].reshape(-1, 1),
            "we2": Wt["We2"].astype(ml_dtypes.bfloat16),
            "be2_r": Wt["be2"].reshape(1, -1),
            "wpack": wpack,
            "bpack": bpack,
            "bconv_rep": np.tile(Wt["b_conv"].reshape(1, -1), (128, 1)),
            "wp_rep": np.tile(Wt["Wp"].reshape(1, -1), (RO_W, 1)),
            "bp_rep": np.tile(Wt["bp"].reshape(1, 1), (RO_W, 1)),
        })
        per_core.append((ns, ne, nlc))

    if "prog" not in _PROGRAM_CACHE:
        _PROGRAM_CACHE["prog"] = _build_program()
    nc = _PROGRAM_CACHE["prog"]

    import time as _time
    _t0 = _time.time()
    res = run_bass_kernel_spmd(nc, in_maps, list(range(NCORE))).results
    globals()["_LAST_RUN_S"] = _time.time() - _t0
    _LAST_RESULTS = res

    feat = np.empty((N_NODES, FEAT), np.float32)
    ratio = np.empty((N_GRAPHS, 1), np.float32)
    for c, (ns, ne, nlc) in enumerate(per_core):
        feat[ns:ne] = res[c]["feat_out"][:nlc]
        ratio[c * GPC:(c + 1) * GPC] = res[c]["ratio_out"][:GPC]
    return feat, ratio


# revision 31
# speedup vs baseline: 1.2414x; 1.1763x over previous
"""CGNet (NNConv + GRU message passing) Trainium2 kernel, 8-way graph-parallel.

Sharding: graphs (and their nodes via the sorted batch vector) are
partitioned across 8 cores; edges live on the core owning their dst node.
Per-edge weight blocks Wedge = edgeMLP(edge_attr) are built once on device
(TensorE, bf16) and staged in HBM. Each iteration: AllGather the node
table, dma_gather out[src] rows, multiply Wedge tiles by the replicated
source vectors on VectorE, then reduce over the input dim AND scatter to
per-tile dst slots in one TensorE pass (one-hot stationary + PSUM revisit
accumulation), dma_scatter_add unique dst rows into an HBM agg table
pre-filled with b_conv, and run the GRU cell per node tile in fp32.
"""
import sys

sys.path.insert(0, "/opt/trn_rl_repo")

import numpy as np
import ml_dtypes

import concourse.bass as bass
import concourse.mybir as mybir
import concourse.tile as tile
from concourse import bacc
from concourse.bass_utils import run_bass_kernel_spmd
from concourse.masks import make_identity

f32 = mybir.dt.float32
bf16 = mybir.dt.bfloat16
i16 = mybir.dt.int16
AF = mybir.ActivationFunctionType
ALU = mybir.AluOpType

N_NODES = 25000
N_EDGES = 50000
NUM_ATOMS = 26
HID = 64
EMB = 64
N_GRAPHS = 64
NUM_NN_ITER = 3
NCORE = 8
GPC = N_GRAPHS // NCORE

NT = 28
NPAD = NT * 128              # 3584 padded nodes per core
ET = 52
EPAD = ET * 128              # 6656 padded edges per core
NTAB = NPAD * NCORE
AGG_T = NT + 1               # +1 trash tile for pad slots
AGG_ROWS = AGG_T * 128
IDXC = EPAD // 16
RO_W = 16
FEAT = EMB + NUM_ATOMS


def _wrap_idx(idx):
    a = np.asarray(idx, dtype=np.int64)
    assert a.shape[0] == EPAD
    w = a.reshape(IDXC, 16).T.astype(np.int16)
    return np.tile(w, (8, 1))


def _tile_major(rows, ntiles):
    d = rows.shape[1]
    return np.ascontiguousarray(rows.reshape(ntiles, 128, d).transpose(1, 0, 2))


def _build_program(phases=4):
    nc = bacc.Bacc("TRN2", target_bir_lowering=False, debug=False,
                   num_devices=NCORE)

    def par(name, shape, dtype, out=False):
        return nc.declare_dram_parameter(name, list(shape), dtype, isOutput=out)

    eaT_in = par("eaT", (4, EPAD), f32)
    idx_in = par("idx2", (128, 2 * IDXC), i16)     # [gather | scatter]
    srel_in = par("srel", (128, ET, 128), bf16)
    xT_in = par("xT", (NUM_ATOMS, NPAD), f32)
    xloc_in = par("xloc", (128, NT, NUM_ATOMS), f32)
    sb_in = par("sbro", (128, NT, RO_W), f32)
    win_in = par("w_in", (NUM_ATOMS, HID), f32)
    we1_in = par("we1", (4, 128), f32)
    be1_in = par("be1_c", (128, 1), f32)
    we2_in = par("we2", (128, 4096), bf16)
    be2_in = par("be2_r", (1, 4096), f32)
    wpack_in = par("wpack", (HID, 576), f32)       # [Wroot|Wo1|Wo2|WihT|WhhT]
    bpack_in = par("bpack", (1, 576), f32)         # [b_in|b_ih|b_hh|bo1|bo2]
    bconv_in = par("bconv_rep", (128, HID), f32)
    wp_in = par("wp_rep", (RO_W, FEAT), f32)
    bp_in = par("bp_rep", (RO_W, 1), f32)

    feat_out = par("feat_out", (NPAD, FEAT), f32, out=True)
    ratio_out = par("ratio_out", (RO_W, 1), f32, out=True)
    dbg_out = par("dbg_h", (NPAD, HID), f32, out=True)

    wedge_hbm = nc.dram_tensor("wedge_hbm", [ET, 128, 4096], bf16)
    local_out = nc.dram_tensor("local_out", [NPAD, HID], f32)
    table = nc.dram_tensor("table", [NTAB, HID], f32, addr_space="Shared")
    agg_hbm = nc.dram_tensor("agg_hbm", [AGG_ROWS, HID], f32)

    def node_rows_ap(dram, ntiles, d):
        return bass.AP(tensor=dram, offset=0,
                       ap=[[d, 128], [128 * d, ntiles], [1, d]])

    with tile.TileContext(nc) as tc:
        with (
            tc.tile_pool(name="persist", bufs=1) as P,
            tc.tile_pool(name="wts", bufs=1) as W,
            tc.tile_pool(name="loop", bufs=2) as L,
            tc.tile_pool(name="sm2", bufs=2) as S,
            tc.tile_pool(name="sc8", bufs=8) as SC,
        ):
            def load(pool, param, shape, dtype):
                t = pool.tile(list(shape), dtype, tag=param.name)
                nc.sync.dma_start(t[:], param[:])
                return t

            B_cm = tc.tile_pool(name="build", bufs=1)
            B = B_cm.__enter__()
            xT = load(B, xT_in, (NUM_ATOMS, NPAD), f32)
            w_in = load(B, win_in, (NUM_ATOMS, HID), f32)
            we1 = load(B, we1_in, (4, 128), f32)
            be1_c = load(B, be1_in, (128, 1), f32)
            we2 = load(B, we2_in, (128, 4096), bf16)
            be2_r = load(B, be2_in, (1, 4096), f32)
            wpack = load(W, wpack_in, (HID, 576), f32)
            bpack = load(W, bpack_in, (1, 576), f32)
            w_root = wpack[:, 0:64]
            wo1 = wpack[:, 64:128]
            wo2 = wpack[:, 128:192]
            w_ihT = wpack[:, 192:384]
            w_hhT = wpack[:, 384:576]
            b_in_r = bpack[:, 0:64]
            b_ih_r = bpack[:, 64:256]
            b_hh_r = bpack[:, 256:448]
            bo1_r = bpack[:, 448:512]
            bo2_r = bpack[:, 512:576]

            # constants: [0:128]=identity, col 128 = ones col, row0[132:260] = ones row
            bigc = P.tile([128, 260], f32)
            nc.gpsimd.memset(bigc[:], 0.0)
            make_identity(nc, bigc[:, 0:128], nomemset=True)
            nc.gpsimd.memset(bigc[:, 128:129], 1.0)
            nc.gpsimd.memset(bigc[0:1, 132:260], 1.0)
            ident = bigc[:, 0:128]
            ones_c = bigc[:, 128:129]
            ones_r = bigc[0:1, 132:260]

            h_a = P.tile([128, NT, HID], f32)

            # ===== phase 1: h2T and initial node state =====
            psA_cm = tc.tile_pool(name="psA", bufs=2, space="PSUM")
            PSA = psA_cm.__enter__()
            h2T = B.tile([128, EPAD], bf16)
            for j in range(EPAD // 512 if phases >= 0.7 else 0):
                eas = B.tile([4, 512], f32, tag="eas")
                nc.sync.dma_start(eas[:], eaT_in[:, j * 512:(j + 1) * 512])
                ph = PSA.tile([128, 512], f32, tag="ph2")
                nc.tensor.matmul(ph[:], we1[:], eas[:], start=True, stop=True)
                nc.scalar.activation(h2T[:, j * 512:(j + 1) * 512], ph[:],
                                     AF.Relu, bias=be1_c[:])

            for t in range(NT if phases >= 0.9 else 0):
                p0 = PSA.tile([128, HID], f32, tag="p0")
                nc.tensor.matmul(p0[:], xT[:, t * 128:(t + 1) * 128], w_in[:],
                                 start=True, stop=False)
                nc.tensor.matmul(p0[:], ones_r, b_in_r, start=False, stop=True)
                nc.scalar.activation(h_a[:, t, :], p0[:], AF.Relu)
            nc.sync.dma_start(node_rows_ap(local_out, NT, HID), h_a[:])
            psA_cm.__exit__(None, None, None)

            # ===== phase 2: Wedge build =====
            psB_cm = tc.tile_pool(name="psB", bufs=2, space="PSUM")
            PSB = psB_cm.__enter__()
            for t in range(ET if phases >= 2 else 0):
                wsb = L.tile([128, 4096], bf16, tag="wt")
                for half in range(2):
                    pw = PSB.tile([128, 2048], f32, tag="pw")
                    for j in range(4):
                        c = half * 2048 + j * 512
                        nc.tensor.matmul(
                            pw[:, j * 512:(j + 1) * 512], ones_r,
                            be2_r[:, c:c + 512],
                            start=True, stop=False)
                        nc.tensor.matmul(
                            pw[:, j * 512:(j + 1) * 512],
                            h2T[:, t * 128:(t + 1) * 128],
                            we2[:, c:c + 512],
                            start=False, stop=True)
                    if half == 0:
                        nc.scalar.copy(wsb[:, 0:2048], pw[:])
                    else:
                        nc.vector.tensor_copy(wsb[:, 2048:4096], pw[:])
                nc.sync.dma_start(wedge_hbm[t], wsb[:])
            psB_cm.__exit__(None, None, None)
            B_cm.__exit__(None, None, None)

            # ===== iterations =====
            if phases >= 2.2:
                idx2 = load(P, idx_in, (128, 2 * IDXC), i16)
                gidx = idx2[:, 0:IDXC]
                sidx = idx2[:, IDXC:2 * IDXC]
                srel = load(P, srel_in, (128, ET, 128), bf16)
                bconv = load(W, bconv_in, (128, HID), f32)
                h_b = P.tile([128, NT, HID], f32)
                msgred = P.tile([128, ET, HID], f32)
                gat = P.tile([128, ET, HID], f32)
                agg = P.tile([128, NT, HID], f32)
                agg_init = P.tile([128, AGG_T, HID], f32)
                rep_in = bass.AP(tensor=bconv[:].tensor,
                                 offset=bconv[:].offset,
                                 ap=[bconv[:].ap[0], [0, AGG_T], [1, HID]])
                nc.vector.tensor_copy(agg_init[:], rep_in)

            psS_cm = tc.tile_pool(name="psS", bufs=2, space="PSUM")
            PSS = psS_cm.__enter__()
            psG_cm = tc.tile_pool(name="psG", bufs=4, space="PSUM")
            PSG = psG_cm.__enter__()

            h_cur, h_nxt = h_a, (h_b if phases >= 2.2 else h_a)
            for it in range(NUM_NN_ITER if phases >= 3 else
                            (1 if phases >= 2.2 else 0)):
                nc.gpsimd.collective_compute(
                    "AllGather", ALU.bypass,
                    replica_groups=[list(range(NCORE))],
                    ins=[local_out[:]], outs=[table[:]])
                if phases >= 2.4:
                    for t0 in range(0, ET, 8):
                        nt_ = min(8, ET - t0)
                        nc.gpsimd.dma_gather(
                            out_ap=gat[:, t0:t0 + nt_, :],
                            in_ap=table[:],
                            idxs_ap=idx2[:, t0 * 8:(t0 + nt_) * 8],
                            num_idxs=nt_ * 128, num_idxs_reg=nt_ * 128,
                            elem_size=HID)
                    nc.sync.dma_start(node_rows_ap(agg_hbm, AGG_T, HID),
                                       agg_init[:])
                for t in range(ET if phases >= 2.6 else 0):
                    wtile = L.tile([128, 4096], bf16, tag="wt")
                    nc.sync.dma_start(wtile[:], wedge_hbm[t])
                    dup2 = S.tile([128, 128], bf16, tag="dup2")
                    g_sl = gat[:, t, :]
                    nc.vector.tensor_copy(
                        dup2[:].rearrange("p (i w) -> p i w", w=2),
                        bass.AP(tensor=g_sl.tensor, offset=g_sl.offset,
                                ap=[g_sl.ap[0], [1, HID], [0, 2]]))
                    tmp = L.tile([128, 4096], bf16, tag="tmp")
                    d_ap = dup2[:]
                    nc.vector.tensor_tensor(
                        tmp[:].rearrange("p (i o2 w) -> p i o2 w", o2=32, w=2),
                        wtile[:].rearrange("p (i o2 w) -> p i o2 w", o2=32, w=2),
                        bass.AP(tensor=d_ap.tensor, offset=d_ap.offset,
                                ap=[d_ap.ap[0], [2, HID], [0, 32], [1, 2]]),
                        op=ALU.mult)
                    pss = PSS.tile([128, HID], f32, tag="pss")
                    o_ap = pss[:]
                    rv = bass.AP(tensor=o_ap.tensor, offset=o_ap.offset,
                                 ap=[o_ap.ap[0], [0, 8], [1, HID]])
                    for q in range(8):
                        nc.tensor.matmul(
                            rv, srel[:, t, :],
                            tmp[:, q * 512:(q + 1) * 512],
                            start=(q == 0), stop=(q == 7))
                    nc.scalar.copy(msgred[:, t, :], pss[:])

                tc.strict_bb_all_engine_barrier()
                if phases >= 2.8:
                    for t0 in range(0, ET, 8):
                        nt_ = min(8, ET - t0)
                        nc.gpsimd.dma_scatter_add(
                            out_ap=agg_hbm[:],
                            in_ap=msgred[:, t0:t0 + nt_, :],
                            idxs_ap=idx2[:, IDXC + t0 * 8:IDXC + (t0 + nt_) * 8],
                            num_idxs=nt_ * 128, num_idxs_reg=nt_ * 128,
                            elem_size=HID)
                if phases >= 3:
                    nc.sync.dma_start(agg[:], node_rows_ap(agg_hbm, NT, HID))

                for t in range(NT if phases >= 3 else 0):
                    pth = PSG.tile([128, 3 * HID], f32, tag="gru")
                    nc.tensor.transpose(pth[0:64, 0:128], h_cur[:, t, :], ident)
                    hTt = SC.tile([64, 128], f32, tag="sc")
                    nc.vector.tensor_copy(hTt[:], pth[0:64, 0:128])
                    pg = PSG.tile([128, 3 * HID], f32, tag="gru")
                    nc.tensor.matmul(pg[:, 0:HID], hTt[:], w_root,
                                     start=True, stop=True)
                    pre = SC.tile([128, HID], f32, tag="sc")
                    nc.vector.tensor_add(pre[:], pg[:, 0:HID], agg[:, t, :])
                    m = SC.tile([128, HID], f32, tag="sc")
                    nc.scalar.activation(m[:], pre[:], AF.Relu)
                    pmt = PSG.tile([128, 3 * HID], f32, tag="gru")
                    nc.tensor.transpose(pmt[0:64, 0:128], m[:], ident)
                    mT = SC.tile([64, 128], f32, tag="sc")
                    nc.vector.tensor_copy(mT[:], pmt[0:64, 0:128])

                    pgi = PSG.tile([128, 3 * HID], f32, tag="gru")
                    nc.tensor.matmul(pgi[:], mT[:], w_ihT,
                                     start=True, stop=False)
                    nc.tensor.matmul(pgi[:], ones_r, b_ih_r,
                                     start=False, stop=True)
                    pgh = PSG.tile([128, 3 * HID], f32, tag="gru")
                    nc.tensor.matmul(pgh[:], hTt[:], w_hhT,
                                     start=True, stop=False)
                    nc.tensor.matmul(pgh[:], ones_r, b_hh_r,
                                     start=False, stop=True)
                    gi = SC.tile([128, 3 * HID], f32, tag="sc")
                    nc.vector.tensor_copy(gi[:], pgi[:])

                    trz = SC.tile([128, 2 * HID], f32, tag="sc")
                    nc.vector.tensor_add(trz[:], gi[:, 0:128], pgh[:, 0:128])
                    rz = SC.tile([128, 2 * HID], f32, tag="sc")
                    nc.scalar.activation(rz[:], trz[:], AF.Sigmoid)
                    tn1 = SC.tile([128, HID], f32, tag="sc")
                    nc.vector.tensor_mul(tn1[:], rz[:, 0:HID], pgh[:, 128:192])
                    tn2 = SC.tile([128, HID], f32, tag="sc")
                    nc.vector.tensor_add(tn2[:], tn1[:], gi[:, 128:192])
                    nn_ = SC.tile([128, HID], f32, tag="sc")
                    nc.scalar.activation(nn_[:], tn2[:], AF.Tanh)
                    th1 = SC.tile([128, HID], f32, tag="sc")
                    nc.vector.tensor_sub(th1[:], h_cur[:, t, :], nn_[:])
                    th2 = SC.tile([128, HID], f32, tag="sc")
                    nc.vector.tensor_mul(th2[:], rz[:, HID:128], th1[:])
                    nc.vector.tensor_add(h_nxt[:, t, :], nn_[:], th2[:])

                if it == 0 and phases >= 3:
                    nc.sync.dma_start(node_rows_ap(dbg_out, NT, HID), h_nxt[:])
                if phases >= 3 and it < NUM_NN_ITER - 1:
                    nc.sync.dma_start(node_rows_ap(local_out, NT, HID),
                                      h_nxt[:])
                h_cur, h_nxt = h_nxt, h_cur
            psG_cm.__exit__(None, None, None)
            psS_cm.__exit__(None, None, None)

            # ===== tail =====
            psT_cm = tc.tile_pool(name="psT", bufs=2, space="PSUM")
            PST = psT_cm.__enter__()
            psR_cm = tc.tile_pool(name="psR", bufs=1, space="PSUM")
            PSR = psR_cm.__enter__()
            if phases >= 4:
                xloc = load(P, xloc_in, (128, NT, NUM_ATOMS), f32)
                sbro = load(P, sb_in, (128, NT, RO_W), f32)
                wp = load(W, wp_in, (RO_W, FEAT), f32)
                bp = load(W, bp_in, (RO_W, 1), f32)
                pro = PSR.tile([RO_W, FEAT], f32, tag="pro")
                pcnt = PSR.tile([RO_W, 1], f32, tag="pcnt")
            for t in range(NT if phases >= 4 else 0):
                p2h = PST.tile([128, 128], f32, tag="tl")
                nc.tensor.transpose(p2h[0:64, 0:128], h_cur[:, t, :], ident)
                hTt2 = SC.tile([64, 128], f32, tag="sc")
                nc.vector.tensor_copy(hTt2[:], p2h[0:64, 0:128])
                po1 = PST.tile([128, 128], f32, tag="tl")
                nc.tensor.matmul(po1[:, 0:HID], hTt2[:], wo1,
                                 start=True, stop=False)
                nc.tensor.matmul(po1[:, 0:HID], ones_r, bo1_r,
                                 start=False, stop=True)
                o1 = SC.tile([128, HID], f32, tag="sc")
                nc.scalar.activation(o1[:], po1[:, 0:HID], AF.Relu)
                p1t = PST.tile([128, 128], f32, tag="tl")
                nc.tensor.transpose(p1t[0:64, 0:128], o1[:], ident)
                o1T = SC.tile([64, 128], f32, tag="sc")
                nc.vector.tensor_copy(o1T[:], p1t[0:64, 0:128])
                po2 = PST.tile([128, 128], f32, tag="tl")
                nc.tensor.matmul(po2[:, 0:EMB], o1T[:], wo2,
                                 start=True, stop=False)
                nc.tensor.matmul(po2[:, 0:EMB], ones_r, bo2_r,
                                 start=False, stop=True)
                feat = SC.tile([128, FEAT], f32, tag="sc")
                nc.vector.tensor_copy(feat[:, 0:EMB], po2[:, 0:EMB])
                nc.vector.tensor_copy(feat[:, EMB:FEAT], xloc[:, t, :])
                fsq = SC.tile([128, FEAT], f32, tag="sc")
                nc.scalar.square(fsq[:], feat[:])
                ss = SC.tile([128, 1], f32, tag="sc")
                nc.vector.tensor_reduce(ss[:], fsq[:], mybir.AxisListType.X,
                                        ALU.add)
                nrm = SC.tile([128, 1], f32, tag="sc")
                nc.scalar.sqrt(nrm[:], ss[:])
                nrm2 = SC.tile([128, 1], f32, tag="sc")
                nc.vector.tensor_scalar_max(nrm2[:], nrm[:], 1e-12)
                rinv = SC.tile([128, 1], f32, tag="sc")
                nc.vector.reciprocal(rinv[:], nrm2[:])
                featn = SC.tile([128, FEAT], f32, tag="sc")
                nc.vector.tensor_scalar_mul(featn[:], feat[:], rinv[:])
                nc.sync.dma_start(
                    bass.AP(tensor=feat_out, offset=t * 128 * FEAT,
                            ap=[[FEAT, 128], [1, FEAT]]),
                    featn[:])
                nc.tensor.matmul(pro[:], sbro[:, t, :], featn[:],
                                 start=(t == 0), stop=(t == NT - 1),
                                 skip_group_check=True)
                nc.tensor.matmul(pcnt[:], sbro[:, t, :], ones_c,
                                 start=(t == 0), stop=(t == NT - 1),
                                 skip_group_check=True)

            if phases >= 4:
                cnt = SC.tile([RO_W, 1], f32, tag="sc")
                nc.vector.tensor_scalar_max(cnt[:], pcnt[:], 1.0)
                rc = SC.tile([RO_W, 1], f32, tag="sc")
                nc.vector.reciprocal(rc[:], cnt[:])
                rm = SC.tile([RO_W, FEAT], f32, tag="sc")
                nc.vector.tensor_scalar_mul(rm[:], pro[:], rc[:])
                tt = SC.tile([RO_W, FEAT], f32, tag="sc")
                nc.vector.tensor_mul(tt[:], rm[:], wp[:])
                s1 = SC.tile([RO_W, 1], f32, tag="sc")
                nc.vector.tensor_reduce(s1[:], tt[:], mybir.AxisListType.X,
                                        ALU.add)
                ratio = SC.tile([RO_W, 1], f32, tag="sc")
                nc.scalar.activation(ratio[:], s1[:], AF.Sigmoid, bias=bp[:])
                nc.sync.dma_start(ratio_out[:], ratio[:])
            psR_cm.__exit__(None, None, None)
            psT_cm.__exit__(None, None, None)

    nc.compile()
    return nc


_PROGRAM_CACHE = {}
_LAST_RESULTS = None


def kernel(**inputs):
    global _LAST_RESULTS
    x = np.asarray(inputs["x"], np.float32)
    edge_index = np.asarray(inputs["edge_index"], np.int64)
    edge_attr = np.asarray(inputs["edge_attr"], np.float32)
    batch = np.asarray(inputs["batch"], np.int64)

    Wt = {k: np.asarray(v, np.float32) for k, v in inputs.items()
          if k not in ("x", "edge_index", "edge_attr", "batch")}

    src, dst = edge_index[0], edge_index[1]
    node_start = np.searchsorted(batch, np.arange(0, N_GRAPHS + 1, GPC))
    core_of_node = np.minimum(batch // GPC, NCORE - 1)
    loc_id = np.arange(N_NODES) - node_start[core_of_node]
    gid = core_of_node * NPAD + loc_id

    wpack = np.concatenate(
        [Wt["W_root"], Wt["Wo1"], Wt["Wo2"],
         np.ascontiguousarray(Wt["W_ih"].T),
         np.ascontiguousarray(Wt["W_hh"].T)], axis=1).astype(np.float32)
    bpack = np.concatenate(
        [Wt["b_in"], Wt["b_ih"], Wt["b_hh"], Wt["bo1"], Wt["bo2"]]
    ).reshape(1, -1).astype(np.float32)

    in_maps = []
    per_core = []
    for c in range(NCORE):
        ns, ne = int(node_start[c]), int(node_start[c + 1])
        nlc = ne - ns
        assert nlc <= NPAD, f"core {c} has {nlc} nodes > {NPAD}"

        eidx = np.nonzero(core_of_node[dst] == c)[0]
        d_loc = (dst[eidx] - ns).astype(np.int64)
        order = np.argsort(d_loc, kind="stable")
        eidx = eidx[order]
        d_loc = d_loc[order]

        uniq, counts = np.unique(d_loc, return_counts=True)
        assert len(uniq) == 0 or counts.max() <= 128
        tiles = []
        cur = []
        pos = 0
        for u, cnt in zip(uniq, counts):
            if len(cur) + cnt > 128:
                cur.extend([-1] * (128 - len(cur)))
                tiles.append(cur)
                cur = []
            cur.extend(range(pos, pos + cnt))
            pos += cnt
        if cur:
            cur.extend([-1] * (128 - len(cur)))
            tiles.append(cur)
        assert len(tiles) <= ET, f"core {c}: {len(tiles)} tiles > {ET}"
        while len(tiles) < ET:
            tiles.append([-1] * 128)

        g_ids = np.zeros(EPAD, np.int64)
        s_ids = np.full(EPAD, NPAD, np.int64)     # default -> trash row
        srel = np.zeros((128, ET, 128), np.float32)
        ea_perm = np.zeros((EPAD, 4), np.float32)
        for t, tl in enumerate(tiles):
            slot_of = {}
            for r, p in enumerate(tl):
                if p < 0:
                    continue
                e = eidx[p]
                g_ids[t * 128 + r] = gid[src[e]]
                ea_perm[t * 128 + r] = edge_attr[e]
                d = d_loc[p]
                if d not in slot_of:
                    slot_of[d] = len(slot_of)
                    s_ids[t * 128 + slot_of[d]] = d
                srel[r, t, slot_of[d]] = 1.0

        x_pad = np.zeros((NPAD, NUM_ATOMS), np.float32)
        x_pad[:nlc] = x[ns:ne]
        sbro = np.zeros((NPAD, RO_W), np.float32)
        col = np.full(NPAD, GPC, np.int64)
        col[:nlc] = batch[ns:ne] - c * GPC
        sbro[np.arange(NPAD), col] = 1.0

        in_maps.append({
            "eaT": np.ascontiguousarray(ea_perm.T),
            "idx2": np.concatenate([_wrap_idx(g_ids), _wrap_idx(s_ids)],
                                   axis=1),
            "srel": srel.astype(ml_dtypes.bfloat16),
            "xT": np.ascontiguousarray(x_pad.T),
            "xloc": _tile_major(x_pad, NT),
            "sbro": _tile_major(sbro, NT),
            "w_in": Wt["W_in"],
            "we1": Wt["We1"],
            "be1_c": Wt["be1"].reshape(-1, 1),
            "we2": Wt["We2"].astype(ml_dtypes.bfloat16),
            "be2_r": Wt["be2"].reshape(1, -1),
            "wpack": wpack,
            "bpack": bpack,
            "bconv_rep": np.tile(Wt["b_conv"].reshape(1, -1), (128, 1)),
            "wp_rep": np.tile(Wt["Wp"].reshape(1, -1), (RO_W, 1)),
            "bp_rep": np.tile(Wt["bp"].reshape(1, 1), (RO_W, 1)),
        })
        per_core.append((ns, ne, nlc))

    if "prog" not in _PROGRAM_CACHE:
        _PROGRAM_CACHE["prog"] = _build_program()
    nc = _PROGRAM_CACHE["prog"]

    import time as _time
    _t0 = _time.time()
    res = run_bass_kernel_spmd(nc, in_maps, list(range(NCORE))).results
    globals()["_LAST_RUN_S"] = _time.time() - _t0
    _LAST_RESULTS = res

    feat = np.empty((N_NODES, FEAT), np.float32)
    ratio = np.empty((N_GRAPHS, 1), np.float32)
    for c, (ns, ne, nlc) in enumerate(per_core):
        feat[ns:ne] = res[c]["feat_out"][:nlc]
        ratio[c * GPC:(c + 1) * GPC] = res[c]["ratio_out"][:GPC]
    return feat, ratio
